# revision 71
# baseline (speedup 1.0000x reference)
"""HPGNN message-passing kernel for Trainium2, data-parallel over graphs on
8 NeuronCores.

Math (see reference): per class c the output needs
  H_emb = softmax(hg @ Hc^T) @ Hc
  L_info = mean_e f(sij),   L_l2 = sum(sij^2)/2/G
  L_path = -mean_g log(uk/3 + 1e-8),
    uk = Mdj[g,s,t]/cz(Adj[g,s,t]) + w1*(max_n tmp0 + 1e-8)
         + w2*sqrt(max_n tmp1 + 1e-8)
    tmp0 = u0*mk/cz(a0*ak), tmp1 = u1*mk/cz(a1*ak)
    u0 = Mdj[g,s,:], mk = Mdj[g,:,t], u1 = u0 @ Mdj[g] (a* = Adj versions).

Only row s, column t and the 2-hop pair products of Mdj reach the output, so
the host bins exactly those edges (pure index preprocessing) into padded
dense (ELL) layouts; the device does all floating-point math: the full-sij
info/l2 stream, all ELL reductions, path-loss transcendentals, softmax and
the embedding matmuls. Each core owns G/8 graphs; host sums the per-core
scalar partials (pure unsharding).

Device-side layouts (per core, 32 graphs = 8192 (g,n) bins as [128, 64]
partition-major, partition p = 4*g_local + (n >> 6)):
  ellpack[c] = [128, 64*WAB | 64*WAB | 64*BP | 64*BP] = A-vals, B-vals,
               pair-A vals, pair-C vals
  constpack  = [128, (a0|a1) 128 | (ak|ak) 128 | hgT 32 | h_initT 15]
  smalls     = [3, st-vals GS*BST | stc GS]   (partition = class)
  hblock     = [15, 384] block-diagonal H_init for the fused embedding matmul
"""

import numpy as np

G = 256
M = 256
N = G * M
C = 3
NCORES = 8
GS = G // NCORES            # graphs per core
ROWS = GS * M               # (g, n) bins per core
K = 5
D = 128

F32 = np.float32

# L_info constants: att/0.5 + 1e-6 and (1-att)/0.500001 + 1e-6
T1_SCALE = 2.0
T1_BIAS = 1e-6
T2_SCALE = -1.999996000008
T2_BIAS = 1.999997000008


# ---------------------------------------------------------------------------
# host-side index preprocessing (no float math on payload)
# ---------------------------------------------------------------------------

def _ell_layout(bins, values, n_bins, width):
    order = np.argsort(bins, kind="stable")
    sb = bins[order]
    cnt = np.bincount(bins, minlength=n_bins)
    starts = np.concatenate([[0], np.cumsum(cnt)[:-1]])
    slot = np.arange(len(sb)) - starts[sb]
    out = np.zeros((values.shape[0], n_bins, width), values.dtype)
    out[:, sb, slot] = values[:, order]
    return out


def preprocess(hg, H_init, path_weight, sij, edge_index, batch, s, t):
    E = edge_index.shape[1]
    src = edge_index[0].astype(np.int64)
    dst = edge_index[1].astype(np.int64)
    g = src >> 8

    anchor_s = (np.arange(G, dtype=np.int64) << 8) + s.astype(np.int64)
    anchor_t = (np.arange(G, dtype=np.int64) << 8) + t.astype(np.int64)
    inA = src == anchor_s[g]
    inB = dst == anchor_t[g]

    idxA = np.nonzero(inA)[0]
    idxB = np.nonzero(inB)[0]
    idxST = np.nonzero(inA & inB)[0]

    cntA = np.bincount(dst[idxA], minlength=N)
    cntB = np.bincount(src[idxB], minlength=N)
    WAB = max(int(cntA.max()), int(cntB.max()), 1)
    ellA = _ell_layout(dst[idxA], sij[:, idxA], N, WAB)
    ellB = _ell_layout(src[idxB], sij[:, idxB], N, WAB)

    cntST = np.bincount(g[idxST], minlength=G)
    BST = max(int(cntST.max()), 1)
    ellST = _ell_layout(g[idxST], sij[:, idxST], G, BST)

    # pairs (a in SetA, e2) with dst_a == src_e2, binned by dst_e2
    ordA = np.argsort(dst[idxA], kind="stable")
    A_sorted = idxA[ordA]
    Astarts = np.concatenate([[0], np.cumsum(cntA)[:-1]])

    repC = cntA[src]
    setC = np.nonzero(repC > 0)[0]
    repC = repC[setC]
    pair_e2 = np.repeat(setC, repC)
    grp_start = np.cumsum(repC) - repC
    kk = np.arange(repC.sum()) - np.repeat(grp_start, repC)
    pair_a = A_sorted[Astarts[src[pair_e2]] + kk]

    pair_bin = dst[pair_e2]
    cntP = np.bincount(pair_bin, minlength=N)
    BP = max(int(cntP.max()), 1)
    ellPA = _ell_layout(pair_bin, sij[:, pair_a], N, BP)
    ellPC = _ell_layout(pair_bin, sij[:, pair_e2], N, BP)

    a0 = cntA.astype(F32)
    ak = cntB.astype(F32)
    a1 = cntP.astype(F32)
    stc = cntST.astype(F32)

    pw = np.clip(path_weight.astype(np.float64), 1e-10, 1.0)
    assert E % (NCORES * 128) == 0
    ecols = E // NCORES
    meta = dict(WAB=WAB, BST=BST, BP=BP, E=E, ECOLS=ecols // 128,
                w1=float(pw[1]), w2=float(pw[2]))

    # block-diagonal H for the fused per-class embedding matmul
    hblock = np.zeros((C * K, C * D), F32)
    for c in range(C):
        hblock[c * K:(c + 1) * K, c * D:(c + 1) * D] = H_init[c]

    in_maps = []
    for kcore in range(NCORES):
        nsl = slice(kcore * ROWS, (kcore + 1) * ROWS)
        gsl = slice(kcore * GS, (kcore + 1) * GS)

        def lay(a, w):
            return a[:, nsl, :].reshape(C, 128, 64 * w)

        def layc(a):
            return a[nsl].reshape(128, 64)

        ellpack = np.concatenate(
            [lay(ellA, WAB), lay(ellB, WAB), lay(ellPA, BP), lay(ellPC, BP)],
            axis=2).astype(np.float16)
        # one mega const tensor: counts | hgT | h_initT | hblock | smalls
        cw = 256 + GS + C * K
        constpack = np.zeros((128, cw + C * D + GS * BST + GS), F32)
        constpack[:, 0:cw] = np.concatenate(
            [layc(a0), layc(a1), layc(ak), layc(ak),
             hg[gsl].T.reshape(128, GS),
             H_init.transpose(2, 0, 1).reshape(128, C * K)], axis=1)
        constpack[0:C * K, cw:cw + C * D] = hblock
        constpack[0:C, cw + C * D:] = np.concatenate(
            [ellST[:, gsl, :].reshape(C, GS * BST),
             np.broadcast_to(stc[gsl], (C, GS))], axis=1)

        in_maps.append(dict(
            sij_chunk=np.ascontiguousarray(
                sij[:, kcore * ecols:(kcore + 1) * ecols]).astype(np.float16),
            ellpack=np.ascontiguousarray(ellpack),
            constpack=np.ascontiguousarray(constpack),
        ))
    return in_maps, meta


def combine(results, meta):
    E = meta["E"]
    h_out = np.concatenate(
        [np.stack([r["h_out"][:, c * D:(c + 1) * D] for c in range(C)])
         for r in results], axis=1)
    p_ln = np.stack([r["pout"][:, 0] for r in results]).sum(
        axis=0, dtype=np.float64)                       # [C]
    p_all = np.stack([r["pout"][0, 1:12] for r in results]).sum(
        axis=0, dtype=np.float64)
    p_sums = p_all[0:9].reshape(C, 3)                   # t2, x2, xd per class
    sum_t2, sum_x2, sum_xd = (p_sums[:, 0].copy(), p_sums[:, 1].copy(),
                              p_sums[:, 2].copy())
    sum_t2[C - 1] += p_all[9]                           # second-half accums
    sum_xd[C - 1] += p_all[10]
    L_info = F32(((sum_xd + sum_t2) / E).mean())
    L_l2 = F32((sum_x2 / 2.0 / G).mean())
    L_path = F32((-p_ln / G).mean())
    return (h_out.astype(F32), L_info, L_path, L_l2)


# ---------------------------------------------------------------------------
# device kernel
# ---------------------------------------------------------------------------

def split_multi_waits(nc, mybir, max_waits=1):
    """This walrus build accepts at most one semaphore wait per instruction;
    hoist extra waits onto preceding same-engine NOPs (equivalent: the
    engine's sequencer blocks on each in turn)."""
    fn = nc.m.functions[0]
    for blk in fn.blocks:
        out = []
        changed = False
        for inst in blk.instructions:
            si = inst.sync_info
            if si is not None and si.on_wait is not None and len(si.on_wait) > max_waits:
                waits = list(si.on_wait)
                si.on_wait = waits[-max_waits:]
                nfront = len(waits) - max_waits
                for i in range(0, nfront, max_waits):
                    nop = mybir.InstNoOp(
                        name=f"{inst.name}_hw{i}",
                        engine=inst.engine,
                        bass_nofuse=True,
                        sync_info=mybir.SyncInfo(
                            on_wait=waits[i:i + max_waits], on_update=[]),
                    )
                    out.append(nop)
                changed = True
            out.append(inst)
        if changed:
            blk.instructions = out


def build_kernel(meta):
    import concourse.bass as bass
    import concourse.mybir as mybir
    from concourse.masks import make_identity
    from concourse.tile import TileContext

    WAB, BST, BP = meta["WAB"], meta["BST"], meta["BP"]
    ECOLS = meta["ECOLS"]
    w1, w2 = meta["w1"], meta["w2"]
    f32 = mybir.dt.float32
    AX = mybir.AxisListType.X
    OP = mybir.AluOpType
    ACTF = mybir.ActivationFunctionType

    OFF_B = 64 * WAB          # ell_b column offset
    OFF_PA = 2 * 64 * WAB
    OFF_PC = OFF_PA + 64 * BP
    EPKW = OFF_PC + 64 * BP

    nc = bass.Bass("TRN2", target_bir_lowering=False, debug=False)

    CW = 256 + GS + C * K            # const cols before hblock
    CW2 = CW + C * D                 # cols before smalls
    CPW = CW2 + GS * BST + GS

    f16 = mybir.dt.float16
    sij = nc.dram_tensor("sij_chunk", [C, 128 * ECOLS], f16,
                         kind="ExternalInput")
    epk_d = nc.dram_tensor("ellpack", [C, 128, EPKW], f16,
                           kind="ExternalInput")
    cpk_d = nc.dram_tensor("constpack", [128, CPW], f32,
                           kind="ExternalInput")

    hout_d = nc.dram_tensor("h_out", [GS, C * D], f32, kind="ExternalOutput")
    pout_d = nc.dram_tensor("pout", [C, 12], f32, kind="ExternalOutput")

    with TileContext(nc) as tc:
        with (
            tc.tile_pool(name="const", bufs=1) as cp,
            tc.tile_pool(name="big", bufs=2) as bp,
            tc.tile_pool(name="ell", bufs=2) as ep,
            tc.tile_pool(name="small", bufs=2) as sp,
            tc.tile_pool(name="psum", bufs=1, space="PSUM") as pp,
        ):
            # ===== DMAs in dependency-priority order: constpack gates the
            # whole small-op graph, then the big streams.
            cpk = cp.tile([128, CPW], f32, tag="cpk")
            nc.sync.dma_start(out=cpk[:], in_=cpk_d[:])
            xs = []
            epks = []
            for c in range(C):
                x = bp.tile([128, ECOLS], f16, tag=f"x{c}", name=f"x{c}")
                nc.sync.dma_start(
                    out=x[:], in_=sij[c].rearrange("(p f) -> p f", p=128))
                xs.append(x)
                epk = ep.tile([128, EPKW], f16, tag=f"epk{c}", name=f"epk{c}")
                nc.sync.dma_start(out=epk[:], in_=epk_d[c])
                epks.append(epk)

            ident = cp.tile([128, 128], f32, tag="ident")
            make_identity(nc, ident[:])
            ones = cp.tile([128, 1], f32, tag="ones")
            nc.vector.memset(ones[:], 1.0)
            # activation bias constants as plain tiles
            bias1 = cp.tile([128, 1], f32, tag="bias1")
            nc.vector.memset(bias1[:], T1_BIAS)
            bias2 = cp.tile([128, 1], f32, tag="bias2")
            nc.vector.memset(bias2[:], T2_BIAS)
            bias8 = cp.tile([128, 1], f32, tag="bias8")
            nc.vector.memset(bias8[:], 1e-8)
            # ln(uk/3 + 1e-8) with the w1*1e-8 epsilon folded in
            biasL = cp.tile([128, 1], f32, tag="biasL")
            nc.vector.memset(biasL[:], 1e-8 + w1 * 1e-8 / 3.0)

            # ===== H embedding: independent of the edge streams, fills the
            # pipeline while the big DMAs are in flight
            hbl = cpk[0:C * K, CW:CW + C * D]
            psc = pp.tile([GS, C * K], f32, tag="psc")
            nc.tensor.matmul(psc[:], lhsT=cpk[:, 256:256 + GS],
                             rhs=cpk[:, 256 + GS:256 + GS + C * K],
                             start=True, stop=True)
            sc = sp.tile([GS, C * K], f32, tag="sc")
            nc.scalar.copy(out=sc[:], in_=psc[:])
            mx = sp.tile([GS, C], f32, tag="mx")
            nc.vector.tensor_reduce(
                out=mx[:], in_=sc[:].rearrange("p (c k) -> p c k", k=K),
                axis=AX, op=OP.max)
            sb = sp.tile([GS, C * K], f32, tag="sb")
            for c in range(C):
                nc.gpsimd.tensor_scalar(
                    out=sb[:, K * c:K * (c + 1)],
                    in0=sc[:, K * c:K * (c + 1)],
                    scalar1=mx[:, c:c + 1], scalar2=None, op0=OP.subtract)
            ex = sp.tile([GS, C * K], f32, tag="ex")
            nc.scalar.activation(ex[:], sb[:], ACTF.Exp)
            se = sp.tile([GS, C], f32, tag="se")
            nc.vector.tensor_reduce(
                out=se[:], in_=ex[:].rearrange("p (c k) -> p c k", k=K),
                axis=AX, op=OP.add)
            rec = sp.tile([GS, C], f32, tag="rec")
            nc.vector.reciprocal(out=rec[:], in_=se[:])
            att = sp.tile([GS, C * K], f32, tag="att")
            for c in range(C):
                nc.gpsimd.tensor_scalar(
                    out=att[:, K * c:K * (c + 1)],
                    in0=ex[:, K * c:K * (c + 1)],
                    scalar1=rec[:, c:c + 1], scalar2=None, op0=OP.mult)
            psT = pp.tile([C * K, GS], f32, tag="psT")
            nc.tensor.transpose(out=psT[:], in_=att[:],
                                identity=ident[:GS, :GS])
            attT = sp.tile([C * K, GS], f32, tag="attT")
            nc.scalar.copy(out=attT[:], in_=psT[:])
            psH = pp.tile([GS, C * D], f32, tag="psH")
            nc.tensor.matmul(psH[:], lhsT=attT[:], rhs=hbl,
                             start=True, stop=True)
            ho = sp.tile([GS, C * D], f32, tag="ho")
            nc.scalar.copy(out=ho[:], in_=psH[:])
            nc.sync.dma_start(out=hout_d[:], in_=ho[:])

            # ---- structural denominators: den = 1/cz((a0|a1) * (ak|ak))
            den = cp.tile([128, 128], f32, tag="den")
            nc.vector.tensor_tensor(out=den[:], in0=cpk[:, 0:128],
                                    in1=cpk[:, 128:256], op=OP.mult)
            nc.vector.tensor_scalar_max(den[:], den[:], 1e-8)
            nc.vector.reciprocal(out=den[:], in_=den[:])

            # ---- st sums and 1/cz(stc), vectorized over classes
            stsum = cp.tile([C, GS], f32, tag="stsum")
            nc.vector.tensor_reduce(
                out=stsum[:],
                in_=cpk[0:C, CW2:CW2 + GS * BST].rearrange(
                    "p (g b) -> p g b", b=BST),
                axis=AX, op=OP.add)
            strec = cp.tile([C, GS], f32, tag="strec")
            nc.vector.tensor_scalar_max(
                strec[:], cpk[0:C, CW2 + GS * BST:CPW], 1e-8)
            nc.vector.reciprocal(out=strec[:], in_=strec[:])
            # uk0 term computed early; the tail only adds the max terms
            uk = sp.tile([C, GS], f32, tag="uk")
            nc.vector.tensor_tensor(out=uk[:], in0=stsum[:], in1=strec[:],
                                    op=OP.mult)

            # per-partition stat accumulators (per class: t2, x2, xd)
            stats = cp.tile([128, 11], f32, tag="stats")
            pm0 = cp.tile([128, 3], f32, tag="pm0")
            pm1 = cp.tile([128, 3], f32, tag="pm1")

            def emit_big(c):
                # ===== big stream: L_info / L_l2 =====
                # The last class is the kernel tail: process it in two
                # column halves so ACT/DVE pipeline instead of serializing.
                x = xs[c]
                t1 = bp.tile([128, ECOLS], f32, tag="t1")
                t2 = bp.tile([128, ECOLS], f32, tag="t2")
                d = bp.tile([128, ECOLS], f32, tag="d")
                if c == C - 1:
                    halves = [(0, ECOLS // 2, 3 * c + 0, 3 * c + 2),
                              (ECOLS // 2, ECOLS, 9, 10)]
                else:
                    halves = [(0, ECOLS, 3 * c + 0, 3 * c + 2)]
                for hidx, (lo, hi, col_t2, col_xd) in enumerate(halves):
                    nc.scalar.activation(t1[:, lo:hi], x[:, lo:hi], ACTF.Ln,
                                         bias=bias1[:], scale=T1_SCALE)
                    nc.scalar.activation(
                        t2[:, lo:hi], x[:, lo:hi], ACTF.Ln, bias=bias2[:],
                        scale=T2_SCALE,
                        accum_out=stats[:, col_t2:col_t2 + 1])
                    # alternate engines across the two tail halves so the
                    # second half's subtract isn't queued behind the first
                    d_eng = (nc.gpsimd if (c == C - 1 and hidx == 0)
                             else nc.vector)
                    d_eng.tensor_tensor(out=d[:, lo:hi], in0=t1[:, lo:hi],
                                        in1=t2[:, lo:hi], op=OP.subtract)
                    nc.vector.scalar_tensor_tensor(
                        out=t1[:, lo:hi], in0=x[:, lo:hi], scalar=1.0,
                        in1=d[:, lo:hi], op0=OP.mult, op1=OP.mult,
                        accum_out=stats[:, col_xd:col_xd + 1])

            def emit_ell(c):
                # ===== ELL stage =====
                epk = epks[c]
                uu = ep.tile([128, 128], f32, tag="uu")    # u0 | mk
                nc.vector.tensor_reduce(
                    out=uu[:],
                    in_=epk[:, 0:OFF_PA].rearrange("p (q b) -> p q b", b=WAB),
                    axis=AX, op=OP.add)
                prod = ep.tile([128, 64 * BP], f32, tag="prod")
                nc.gpsimd.tensor_tensor(
                    out=prod[:], in0=epk[:, OFF_PA:OFF_PC],
                    in1=epk[:, OFF_PC:EPKW], op=OP.mult)
                u1 = ep.tile([128, 64], f32, tag="u1")
                nc.vector.tensor_reduce(
                    out=u1[:], in_=prod[:].rearrange("p (q b) -> p q b", b=BP),
                    axis=AX, op=OP.add)
                num = ep.tile([128, 128], f32, tag="num")  # u0*mk | u1*mk
                nc.gpsimd.tensor_tensor(out=num[:, 0:64], in0=uu[:, 0:64],
                                        in1=uu[:, 64:128], op=OP.mult)
                nc.gpsimd.tensor_tensor(out=num[:, 64:128], in0=u1[:],
                                        in1=uu[:, 64:128], op=OP.mult)
                tmp = ep.tile([128, 128], f32, tag="tmp")
                nc.gpsimd.tensor_tensor(out=tmp[:], in0=num[:], in1=den[:],
                                        op=OP.mult)
                nc.vector.tensor_reduce(out=pm0[:, c:c + 1],
                                        in_=tmp[:, 0:64], axis=AX, op=OP.max)
                nc.vector.tensor_reduce(out=pm1[:, c:c + 1],
                                        in_=tmp[:, 64:128], axis=AX, op=OP.max)

            for c in range(C):
                if c == C - 1:
                    emit_ell(c)
                    emit_big(c)
                else:
                    emit_big(c)
                    emit_ell(c)
            # sum(x*x): ACT Square with accumulate, emitted last so the
            # scheduler favors the Ln chain
            for c in range(C):
                sq = bp.tile([128, ECOLS], f32, tag="sq", name=f"sq{c}")
                if c == 0:
                    nc.vector.scalar_tensor_tensor(
                        out=sq[:], in0=xs[c][:], scalar=1.0, in1=xs[c][:],
                        op0=OP.mult, op1=OP.mult,
                        accum_out=stats[:, 3 * c + 1:3 * c + 2])
                else:
                    nc.scalar.activation(
                        sq[:], xs[c][:], ACTF.Square,
                        accum_out=stats[:, 3 * c + 1:3 * c + 2])

            # ---- cross-partition combines
            ps0 = pp.tile([3, 128], f32, tag="ps0")
            nc.tensor.transpose(out=ps0[:], in_=pm0[:], identity=ident[:])
            r0 = sp.tile([3, 128], f32, tag="r0")
            nc.scalar.copy(out=r0[:], in_=ps0[:])
            m0 = sp.tile([3, GS], f32, tag="m0")
            nc.vector.tensor_reduce(
                out=m0[:], in_=r0[:].rearrange("p (g q) -> p g q", q=4),
                axis=AX, op=OP.max)
            lnp = sp.tile([128, 3], f32, tag="lnp")
            nc.scalar.activation(lnp[:], pm1[:], ACTF.Ln, bias=bias8[:])
            sqp = sp.tile([128, 3], f32, tag="sqp")
            nc.scalar.activation(sqp[:], lnp[:], ACTF.Exp, scale=0.5)
            ps1 = pp.tile([3, 128], f32, tag="ps1")
            nc.tensor.transpose(out=ps1[:], in_=sqp[:], identity=ident[:])
            r1 = sp.tile([3, 128], f32, tag="r1")
            nc.scalar.copy(out=r1[:], in_=ps1[:])
            m1 = sp.tile([3, GS], f32, tag="m1")
            nc.vector.tensor_reduce(
                out=m1[:], in_=r1[:].rearrange("p (g q) -> p g q", q=4),
                axis=AX, op=OP.max)

            psS = pp.tile([1, 11], f32, tag="psS")
            nc.tensor.matmul(psS[:], lhsT=ones[:], rhs=stats[:],
                             start=True, stop=True)
            po = sp.tile([C, 12], f32, tag="po")
            nc.vector.memset(po[:], 0.0)
            nc.scalar.copy(out=po[0:1, 1:12], in_=psS[:])

            # ---- uk chain, vectorized over classes [3, GS]
            # uk = uk0 + w1*m0 + w2*exp(0.5*ln(m1 + 1e-8));
            # loss term ln(uk/3 + 1e-8 + w1*1e-8/3) folds the w0 epsilon
            nc.vector.scalar_tensor_tensor(
                out=uk[:], in0=m0[:], scalar=w1, in1=uk[:],
                op0=OP.mult, op1=OP.add)
            nc.vector.scalar_tensor_tensor(
                out=uk[:], in0=m1[:], scalar=w2, in1=uk[:],
                op0=OP.mult, op1=OP.add)
            lnuk = sp.tile([C, GS], f32, tag="lnuk")
            nc.scalar.activation(lnuk[:], uk[:], ACTF.Ln, bias=biasL[:C, :],
                                 scale=1.0 / 3.0, accum_out=po[:, 0:1])
            nc.sync.dma_start(out=pout_d[:], in_=po[:])

    return nc


# ---------------------------------------------------------------------------
# entry point
# ---------------------------------------------------------------------------

def kernel(hg, H_init, path_weight, sij, edge_index, batch, s, t):
    from concourse import bass_utils
    import concourse.mybir as mybir

    in_maps, meta = preprocess(
        np.asarray(hg), np.asarray(H_init), np.asarray(path_weight),
        np.ascontiguousarray(sij, dtype=np.float32),
        np.asarray(edge_index), np.asarray(batch), np.asarray(s),
        np.asarray(t))
    nc = build_kernel(meta)
    split_multi_waits(nc, mybir)
    res = bass_utils.run_bass_kernel_spmd(
        nc, in_maps, list(range(NCORES)), trace=False)
    return combine(res.results, meta)
